# revision 17
# baseline (speedup 1.0000x reference)
"""Trainium2 Bass kernel for ContextualAttention (two_input=False path).

Math (B=128, C=512, n_iter=128, per iteration n):
    scores[n,b,o,0] = 10 * sum_c mid[b,c,2n]   * left_cat[o,c,2n+1]
    scores[n,b,o,1] = 10 * sum_c (mid[b,c,2n]*left_cat[o,c,2n]
                                  + mid[b,c,2n+1]*left_cat[o,c,2n+1])
    att = softmax(scores, axis=o)                                # [n,B,128,2]
    out0[b,c,3n+t] = att[n,b,c,t] (c<128, else 0); out0[b,c,3n+2] = sc00[b,c,n]
    out1 same with sc10. sc01/sc11 unused.

Sharding: data-parallel over n, 16 iterations per core (core k owns the
l-window [32k, 32k+32) of mid/left_cat).

Bottleneck model (from NTFF traces): the 16 per-core DMA engines cap at
~25 GB/s each (~400 GB/s aggregate), and the PE is moving-operand
fetch-bound at ~2 B/cycle, so both wire bytes and fetched bytes matter.

  - Wire format is 3 B/elem: fp16 hi plane + float8-e5m2 lo plane
    (lo = x - fp16(x), representable unscaled thanks to e5m2's 2^-16
    subnormals).  12.6 MB/core streams in ~33 us vs ~43 us for the fp32
    equivalent.
  - Scores: mh*lh (fp16 matmul) + [ml*lh8 + mh8*ll] (one fp8 DoubleRow
    matmul, two k-tiles per instruction).  mh8/lh8 are e5m2 casts of the
    hi planes done on-device on DVE (its 2x mode runs ~0.7us per
    iteration-slice).  Corrections are ~2^-11 relative so e5m2's coarse
    mantissa still leaves score error ~2^-14; measured end-to-end rel-err
    vs the fp64 reference is ~1.2e-3.
  - The correction accumulates directly into the main PSUM group (no
    scale-back pass, since the lo planes are unscaled).
  - DMA groups are ragged (8,8,8,4,2,2 l's): big chunks amortize
    descriptors early while the last chunks are small, so the final
    DMA -> cast -> matmul -> rowmax -> exp -> output chain after the
    last input byte is short.
  - Casts are emitted one iteration ahead of the row-max on the in-order
    DVE queue, and the PSUM pool is deep (12 bufs), so the PE is gated
    only by input arrival, never by the softmax drain.

Softmax: row-max (negated) via DVE feeds the exp activation bias on
ScalarE; exp writes fp16, and the host divides by the per-row sum (the max
shift cancels) and assembles the full outputs.
"""

import os
from functools import lru_cache

import ml_dtypes
import numpy as np

import concourse.bacc as bacc
import concourse.mybir as mybir
import concourse.tile as tile
from concourse.bass_utils import run_bass_kernel_spmd

N_CORES = 8
B = 128          # batch rows (= out partition) and also conv out channels o
C = 512          # contraction dim
NPC = 16         # iterations n per core
LW = 2 * NPC     # l-window per core (32)
GROUPS = (4, 4, 4, 4, 4, 4, 4, 4)  # l-group sizes (DMA/cast granularity)
NG = len(GROUPS)
GOFF = [sum(GROUPS[:i]) for i in range(NG + 1)]
SCALE = 10.0     # softmax scale, folded into mid on the host

F8 = ml_dtypes.float8_e5m2
F8_DT = mybir.dt.float8e5

# Results of the last run (exec_time_ns etc.), for the local test harness.
last_results = None


def _iter_group(n):
    """l-group index and local l-offset for iteration n (l = 2n)."""
    l = 2 * n
    for g in range(NG):
        if l < GOFF[g + 1]:
            return g, l - GOFF[g]
    raise ValueError(n)


@lru_cache(maxsize=1)
def build_program():
    """One SPMD program; all 8 cores run it on their own shard."""
    nc = bacc.Bacc(None, target_bir_lowering=False, debug=False)
    f32 = mybir.dt.float32
    f16 = mybir.dt.float16

    # Host-prepped layouts, per core:
    #   mh[c, l, b] = fp16(10 * mid[b, c, 32k + l])          [512, 32, 128]
    #   ml[c, l, b] = e5m2(10*mid - mh)                      [512, 32, 128]
    #   lh/ll same for left_cat[o, c, 32k + l]
    mh = nc.dram_tensor("mh", [C, LW, B], f16, kind="ExternalInput")
    ml = nc.dram_tensor("ml", [C, LW, B], F8_DT, kind="ExternalInput")
    lh = nc.dram_tensor("lh", [C, LW, B], f16, kind="ExternalInput")
    ll = nc.dram_tensor("ll", [C, LW, B], F8_DT, kind="ExternalInput")
    # att[b, n'*256 + t*128 + o] = exp(scores - rowmax)   (unnormalized)
    att = nc.dram_tensor("att", [B, NPC * 2 * B], f16, kind="ExternalOutput")

    # [c, cc, l, b] views: partition dim = c within a 128-chunk.
    mh_r = mh[:].rearrange("(cc c) l b -> c cc l b", cc=4)
    ml_r = ml[:].rearrange("(cc c) l b -> c cc l b", cc=4)
    lh_r = lh[:].rearrange("(cc c) l b -> c cc l b", cc=4)
    ll_r = ll[:].rearrange("(cc c) l b -> c cc l b", cc=4)

    DR = mybir.MatmulPerfMode.DoubleRow

    with tile.TileContext(nc) as tc:
        # All input tiles stay resident (96 KiB/partition total),
        # allocated individually (exact ragged sizes); freed LIFO below.
        mh_t, lh_t, m8_t, l8_t, frees = [], [], [], [], []
        for g in range(NG):
            gl = GROUPS[g]
            mhg, f0 = tc.tile([128, 4, gl, B], f16, name=f"mh{g}")
            lhg, f1 = tc.tile([128, 4, gl, B], f16, name=f"lh{g}")
            # fp8 pair tiles for DoubleRow: dim1 packs the two
            # stationary/moving planes contracted by one instruction.
            # m8: [0]=ml (lo, DMA), [1]=mh8 (hi cast)
            # l8: [0]=lh8 (hi cast), [1]=ll (lo, DMA)
            m8g, f2 = tc.tile([128, 2, 4, gl, B], F8_DT, name=f"m8{g}")
            l8g, f3 = tc.tile([128, 2, 4, gl, B], F8_DT, name=f"l8{g}")
            mh_t.append(mhg)
            lh_t.append(lhg)
            m8_t.append(m8g)
            l8_t.append(l8g)
            frees += [f0, f1, f2, f3]
        with (
            tc.tile_pool(name="stat", bufs=8) as stat,
            tc.tile_pool(name="attb", bufs=8) as attb,
            tc.tile_pool(name="ps", bufs=8, space="PSUM") as ps,
        ):
            # Input DMAs in l-group order, on two HWDGE rings: m tensors
            # on SP (idle engine — all issued up front; ring-full blocking
            # is harmless there), l tensors on ACT.  Critically, no up-front input
            # DMA beyond the first two groups is issued from the ACT
            # engine: dma_start blocks when the descriptor ring fills, and
            # anything queued behind it on that engine (the exps) would
            # stall until most of the stream has drained.  Later groups go
            # just-in-time (group g+2 at the start of group g's compute,
            # when g's descriptors have already drained from the ring), so
            # the issue never blocks and the exps behind it never wait.
            def emit_l_dma(g):
                sl = slice(GOFF[g], GOFF[g + 1])
                nc.scalar.dma_start(out=lh_t[g][:], in_=lh_r[:, :, sl, :])
                nc.scalar.dma_start(out=l8_t[g][:, 1], in_=ll_r[:, :, sl, :])

            # group 0's l tensors ride the SP ring: SP starts issuing
            # ~1.5us before ACT (which first loads its activation table),
            # and the first compute is gated on exactly this data.
            sl0 = slice(GOFF[0], GOFF[1])
            nc.sync.dma_start(out=mh_t[0][:], in_=mh_r[:, :, sl0, :])
            nc.sync.dma_start(out=m8_t[0][:, 0], in_=ml_r[:, :, sl0, :])
            nc.sync.dma_start(out=lh_t[0][:], in_=lh_r[:, :, sl0, :])
            nc.sync.dma_start(out=l8_t[0][:, 1], in_=ll_r[:, :, sl0, :])
            for g in range(1, NG):
                sl = slice(GOFF[g], GOFF[g + 1])
                nc.sync.dma_start(out=mh_t[g][:], in_=mh_r[:, :, sl, :])
                nc.sync.dma_start(out=m8_t[g][:, 0], in_=ml_r[:, :, sl, :])
            emit_l_dma(1)
            emit_l_dma(2)
            emit_l_dma(3)

            def emit_casts(n):
                # On-device e5m2 casts of the hi planes for iteration n
                # (DoubleRow needs both operands fp8).  Both on DVE — its
                # 2x mode runs them ~3x faster than ACT Copy.
                g, lo = _iter_group(n)
                lp = slice(lo, lo + 2)
                nc.vector.tensor_copy(
                    out=m8_t[g][:, 1, :, lp, :], in_=mh_t[g][:, :, lp, :])
                nc.vector.tensor_copy(
                    out=l8_t[g][:, 0, :, lp, :], in_=lh_t[g][:, :, lp, :])

            # The DVE program order is c0, c1, r0, c2, r1, ...: iteration
            # n's casts are emitted before iteration n-1's row-max, so the
            # PE (which only waits on casts) stays one iteration ahead of
            # the PE->reduce->cast dependency chain and never drains.
            emit_casts(0)
            group_started = set()
            for n in range(NPC):
                g, lo = _iter_group(n)
                if g not in group_started:
                    group_started.add(g)
                    if g + 4 < NG:
                        emit_l_dma(g + 4)
                mhg, lhg, m8g, l8g = mh_t[g], lh_t[g], m8_t[g], l8_t[g]
                l0, l1 = lo, lo + 1
                s, sub = divmod(n, 2)
                if sub == 0:
                    att_t = attb.tile([B, 4 * B], f16, tag="att")
                # psum [128, 2, 128]: [:,0]=t1 scores, [:,1]=t0 scores
                pab = ps.tile([B, 2, B], f32, tag="ps", name=f"pab{n}")
                for cc in range(4):
                    # fp16 main: stationary M(l0) x moving [L(l0)|L(l1)]
                    # writes [t1 | t0] at once; order mm,DR,DR,mm keeps
                    # ldweights hidden under the moving passes.
                    nc.tensor.matmul(
                        pab[:, :, :], mhg[:, cc, l0, :],
                        lhg[:, cc, l0:l0 + 2, :],
                        start=(cc == 0), stop=False)
                    # fp8 corrections: ml*lh8 + mh8*ll in one DoubleRow
                    nc.tensor.matmul(
                        pab[:, :, :], m8g[:, :, cc, l0, :],
                        l8g[:, :, cc, l0:l0 + 2, :],
                        start=False, stop=False, perf_mode=DR)
                    nc.tensor.matmul(
                        pab[:, 0, :], m8g[:, :, cc, l1, :],
                        l8g[:, :, cc, l1, :],
                        start=False, stop=False, perf_mode=DR)
                    # t1 second term: M(l1) x L(l1)
                    nc.tensor.matmul(
                        pab[:, 0, :], mhg[:, cc, l1, :],
                        lhg[:, cc, l1, :],
                        start=False, stop=(cc == 3))
                if n + 1 < NPC:
                    emit_casts(n + 1)
                # negated row-max of both halves in one DVE pass
                nmx = stat.tile([B, 2, 1], f32, tag="nmx")
                nc.vector.reduce_max(
                    out=nmx[:], in_=pab[:],
                    axis=mybir.AxisListType.X, negate=True)
                for t in range(2):
                    nc.scalar.activation(
                        att_t[:, (2 * sub + t) * B:(2 * sub + t + 1) * B],
                        pab[:, 1 - t, :],
                        mybir.ActivationFunctionType.Exp,
                        bias=nmx[:, 1 - t, 0:1])
                if sub == 1:
                    # output on the idle GPSIMD SWDGE ring so it never
                    # queues behind the input streams
                    nc.gpsimd.dma_start(
                        out=att[:, s * 512:(s + 1) * 512], in_=att_t[:])

        for f in reversed(frees):
            f()

    nc.compile()
    return nc


def _shard_inputs(left, right, mid):
    """Per-core [c, l, b]-contiguous fp16 hi + e5m2 lo shards; folds the
    softmax scale into mid."""
    in_maps = []
    for k in range(N_CORES):
        lo = 32 * k
        if lo < left.shape[2]:
            lsl = left[:, :, lo:lo + LW]
        else:
            lsl = right[:, :, lo - left.shape[2]:lo - left.shape[2] + LW]
        msl = mid[:, :, lo:lo + LW] * np.float32(SCALE)
        msl = np.ascontiguousarray(msl.transpose(1, 2, 0))
        lsl = np.ascontiguousarray(lsl.transpose(1, 2, 0))
        mh = msl.astype(np.float16)
        ml = (msl - mh.astype(np.float32)).astype(F8)
        lh = lsl.astype(np.float16)
        ll = (lsl - lh.astype(np.float32)).astype(F8)
        in_maps.append({"mh": mh, "ml": ml, "lh": lh, "ll": ll})
    return in_maps


def kernel(left, right, mid, sc00, sc01, sc10, sc11):
    global last_results
    left = np.asarray(left, dtype=np.float32)
    right = np.asarray(right, dtype=np.float32)
    mid = np.asarray(mid, dtype=np.float32)
    sc00 = np.asarray(sc00, dtype=np.float32)
    sc10 = np.asarray(sc10, dtype=np.float32)

    nc = build_program()
    in_maps = _shard_inputs(left, right, mid)
    trace = bool(int(os.environ.get("BASS_KERNEL_TRACE", "0")))
    last_results = run_bass_kernel_spmd(
        nc, in_maps, core_ids=list(range(N_CORES)), trace=trace,
    )

    # [k, b, n', t, o]
    att = np.stack([np.asarray(r["att"], dtype=np.float32)
                    for r in last_results.results])
    att = att.reshape(N_CORES, B, NPC, 2, B)
    att = att / att.sum(axis=4, keepdims=True)
    # -> [b, o(=c<128), n = k*NPC + n', t]
    attn = att.transpose(1, 4, 0, 2, 3).reshape(B, B, N_CORES * NPC, 2)

    Ls = sc00.shape[2]
    outs = []
    for sc in (sc00, sc10):
        out = np.zeros((B, C, Ls), np.float32)
        v = out.reshape(B, C, N_CORES * NPC, 3)
        v[:, :B, :, 0:2] = attn
        v[:, :, :, 2] = sc[:, :, :N_CORES * NPC]
        outs.append(out)
    return tuple(outs)


# revision 18
# speedup vs baseline: 1.0266x; 1.0266x over previous
"""Trainium2 Bass kernel for ContextualAttention (two_input=False path).

Math (B=128, C=512, n_iter=128, per iteration n):
    scores[n,b,o,0] = 10 * sum_c mid[b,c,2n]   * left_cat[o,c,2n+1]
    scores[n,b,o,1] = 10 * sum_c (mid[b,c,2n]*left_cat[o,c,2n]
                                  + mid[b,c,2n+1]*left_cat[o,c,2n+1])
    att = softmax(scores, axis=o)                                # [n,B,128,2]
    out0[b,c,3n+t] = att[n,b,c,t] (c<128, else 0); out0[b,c,3n+2] = sc00[b,c,n]
    out1 same with sc10. sc01/sc11 unused.

Sharding: data-parallel over n, 16 iterations per core (core k owns the
l-window [32k, 32k+32) of mid/left_cat).

Bottleneck model (from NTFF traces): the 16 per-core DMA engines cap at
~25 GB/s each (~400 GB/s aggregate), and the PE is moving-operand
fetch-bound at ~2 B/cycle, so both wire bytes and fetched bytes matter.

  - Wire format is 3 B/elem: fp16 hi plane + float8-e5m2 lo plane
    (lo = x - fp16(x), representable unscaled thanks to e5m2's 2^-16
    subnormals).  12.6 MB/core streams in ~33 us vs ~43 us for the fp32
    equivalent.
  - Scores: mh*lh (fp16 matmul) + [ml*lh8 + mh8*ll] (one fp8 DoubleRow
    matmul, two k-tiles per instruction).  mh8/lh8 are e5m2 casts of the
    hi planes done on-device on DVE (its 2x mode runs ~0.7us per
    iteration-slice).  Corrections are ~2^-11 relative so e5m2's coarse
    mantissa still leaves score error ~2^-14; measured end-to-end rel-err
    vs the fp64 reference is ~1.2e-3.
  - The correction accumulates directly into the main PSUM group (no
    scale-back pass, since the lo planes are unscaled).
  - DMA groups are ragged (8,8,8,4,2,2 l's): big chunks amortize
    descriptors early while the last chunks are small, so the final
    DMA -> cast -> matmul -> rowmax -> exp -> output chain after the
    last input byte is short.
  - Casts are emitted one iteration ahead of the row-max on the in-order
    DVE queue, and the PSUM pool is deep (12 bufs), so the PE is gated
    only by input arrival, never by the softmax drain.

Softmax: row-max (negated) via DVE feeds the exp activation bias on
ScalarE; exp writes fp16, and the host divides by the per-row sum (the max
shift cancels) and assembles the full outputs.
"""

import os
from functools import lru_cache

import ml_dtypes
import numpy as np

import concourse.bacc as bacc
import concourse.mybir as mybir
import concourse.tile as tile
from concourse.bass_utils import run_bass_kernel_spmd

N_CORES = 8
B = 128          # batch rows (= out partition) and also conv out channels o
C = 512          # contraction dim
NPC = 16         # iterations n per core
LW = 2 * NPC     # l-window per core (32)
GROUPS = (4, 8, 8, 8, 4)   # ragged l-group sizes (DMA/cast granularity)
NG = len(GROUPS)
GOFF = [sum(GROUPS[:i]) for i in range(NG + 1)]
SCALE = 10.0     # softmax scale, folded into mid on the host

F8 = ml_dtypes.float8_e4m3
F8_DT = mybir.dt.float8e4

# Results of the last run (exec_time_ns etc.), for the local test harness.
last_results = None


def _iter_group(n):
    """l-group index and local l-offset for iteration n (l = 2n)."""
    l = 2 * n
    for g in range(NG):
        if l < GOFF[g + 1]:
            return g, l - GOFF[g]
    raise ValueError(n)


@lru_cache(maxsize=1)
def build_program():
    """One SPMD program; all 8 cores run it on their own shard."""
    nc = bacc.Bacc(None, target_bir_lowering=False, debug=False)
    f32 = mybir.dt.float32
    f16 = mybir.dt.float16

    # Host-prepped layouts, per core:
    #   mh[c, l, b] = fp16(10 * mid[b, c, 32k + l])          [512, 32, 128]
    #   ml[c, l, b] = e5m2(10*mid - mh)                      [512, 32, 128]
    #   lh/ll same for left_cat[o, c, 32k + l]
    mh = nc.dram_tensor("mh", [C, LW, B], f16, kind="ExternalInput")
    ml = nc.dram_tensor("ml", [C, LW, B], F8_DT, kind="ExternalInput")
    lh = nc.dram_tensor("lh", [C, LW, B], f16, kind="ExternalInput")
    ll = nc.dram_tensor("ll", [C, LW, B], F8_DT, kind="ExternalInput")
    # att[b, n'*256 + t*128 + o] = exp(scores - rowmax)   (unnormalized)
    att = nc.dram_tensor("att", [B, NPC * 2 * B], f16, kind="ExternalOutput")

    # [c, cc, l, b] views: partition dim = c within a 128-chunk.
    mh_r = mh[:].rearrange("(cc c) l b -> c cc l b", cc=4)
    ml_r = ml[:].rearrange("(cc c) l b -> c cc l b", cc=4)
    lh_r = lh[:].rearrange("(cc c) l b -> c cc l b", cc=4)
    ll_r = ll[:].rearrange("(cc c) l b -> c cc l b", cc=4)

    DR = mybir.MatmulPerfMode.DoubleRow

    with tile.TileContext(nc) as tc:
        # All input tiles stay resident (96 KiB/partition total),
        # allocated individually (exact ragged sizes); freed LIFO below.
        mh_t, lh_t, m8_t, l8_t, frees = [], [], [], [], []
        for g in range(NG):
            gl = GROUPS[g]
            mhg, f0 = tc.tile([128, 4, gl, B], f16, name=f"mh{g}")
            lhg, f1 = tc.tile([128, 4, gl, B], f16, name=f"lh{g}")
            # fp8 pair tiles for DoubleRow: dim1 packs the two
            # stationary/moving planes contracted by one instruction.
            # m8: [0]=ml (lo, DMA), [1]=mh8 (hi cast)
            # l8: [0]=lh8 (hi cast), [1]=ll (lo, DMA)
            m8g, f2 = tc.tile([128, 2, 4, gl, B], F8_DT, name=f"m8{g}")
            l8g, f3 = tc.tile([128, 2, 4, gl, B], F8_DT, name=f"l8{g}")
            mh_t.append(mhg)
            lh_t.append(lhg)
            m8_t.append(m8g)
            l8_t.append(l8g)
            frees += [f0, f1, f2, f3]
        with (
            tc.tile_pool(name="stat", bufs=8) as stat,
            tc.tile_pool(name="attb", bufs=8) as attb,
            tc.tile_pool(name="ps", bufs=8, space="PSUM") as ps,
        ):
            # Input DMAs in l-group order, on two HWDGE rings: m tensors
            # on SP (idle engine — all issued up front; ring-full blocking
            # is harmless there), l tensors on ACT.  Critically, no up-front input
            # DMA beyond the first two groups is issued from the ACT
            # engine: dma_start blocks when the descriptor ring fills, and
            # anything queued behind it on that engine (the exps) would
            # stall until most of the stream has drained.  Later groups go
            # just-in-time (group g+2 at the start of group g's compute,
            # when g's descriptors have already drained from the ring), so
            # the issue never blocks and the exps behind it never wait.
            def emit_l_dma(g):
                sl = slice(GOFF[g], GOFF[g + 1])
                nc.scalar.dma_start(out=lh_t[g][:], in_=lh_r[:, :, sl, :])
                nc.scalar.dma_start(out=l8_t[g][:, 1], in_=ll_r[:, :, sl, :])

            # group 0's l tensors ride the SP ring: SP starts issuing
            # ~1.5us before ACT (which first loads its activation table),
            # and the first compute is gated on exactly this data.
            sl0 = slice(GOFF[0], GOFF[1])
            nc.sync.dma_start(out=mh_t[0][:], in_=mh_r[:, :, sl0, :])
            nc.sync.dma_start(out=m8_t[0][:, 0], in_=ml_r[:, :, sl0, :])
            nc.sync.dma_start(out=lh_t[0][:], in_=lh_r[:, :, sl0, :])
            nc.sync.dma_start(out=l8_t[0][:, 1], in_=ll_r[:, :, sl0, :])
            for g in range(1, NG):
                sl = slice(GOFF[g], GOFF[g + 1])
                nc.sync.dma_start(out=mh_t[g][:], in_=mh_r[:, :, sl, :])
                nc.sync.dma_start(out=m8_t[g][:, 0], in_=ml_r[:, :, sl, :])
            emit_l_dma(1)
            emit_l_dma(2)
            emit_l_dma(3)

            def emit_casts(n):
                # On-device e5m2 casts of the hi planes for iteration n
                # (DoubleRow needs both operands fp8).  Both on DVE — its
                # 2x mode runs them ~3x faster than ACT Copy.
                g, lo = _iter_group(n)
                lp = slice(lo, lo + 2)
                nc.vector.tensor_copy(
                    out=m8_t[g][:, 1, :, lp, :], in_=mh_t[g][:, :, lp, :])
                nc.vector.tensor_copy(
                    out=l8_t[g][:, 0, :, lp, :], in_=lh_t[g][:, :, lp, :])

            # The DVE program order is c0, c1, r0, c2, r1, ...: iteration
            # n's casts are emitted before iteration n-1's row-max, so the
            # PE (which only waits on casts) stays one iteration ahead of
            # the PE->reduce->cast dependency chain and never drains.
            emit_casts(0)
            group_started = set()
            for n in range(NPC):
                g, lo = _iter_group(n)
                if g not in group_started:
                    group_started.add(g)
                    if g + 4 < NG:
                        emit_l_dma(g + 4)
                    elif g == 1 and NG == 5:
                        emit_l_dma(4)
                mhg, lhg, m8g, l8g = mh_t[g], lh_t[g], m8_t[g], l8_t[g]
                l0, l1 = lo, lo + 1
                s, sub = divmod(n, 2)
                if sub == 0:
                    att_t = attb.tile([B, 4 * B], f16, tag="att")
                # psum [128, 2, 128]: [:,0]=t1 scores, [:,1]=t0 scores
                pab = ps.tile([B, 2, B], f32, tag="ps", name=f"pab{n}")
                for cc in range(4):
                    # fp16 main: stationary M(l0) x moving [L(l0)|L(l1)]
                    # writes [t1 | t0] at once; order mm,DR,DR,mm keeps
                    # ldweights hidden under the moving passes.
                    nc.tensor.matmul(
                        pab[:, :, :], mhg[:, cc, l0, :],
                        lhg[:, cc, l0:l0 + 2, :],
                        start=(cc == 0), stop=False)
                    # fp8 corrections: ml*lh8 + mh8*ll in one DoubleRow
                    nc.tensor.matmul(
                        pab[:, :, :], m8g[:, :, cc, l0, :],
                        l8g[:, :, cc, l0:l0 + 2, :],
                        start=False, stop=False, perf_mode=DR)
                    nc.tensor.matmul(
                        pab[:, 0, :], m8g[:, :, cc, l1, :],
                        l8g[:, :, cc, l1, :],
                        start=False, stop=False, perf_mode=DR)
                    # t1 second term: M(l1) x L(l1)
                    nc.tensor.matmul(
                        pab[:, 0, :], mhg[:, cc, l1, :],
                        lhg[:, cc, l1, :],
                        start=False, stop=(cc == 3))
                if n + 1 < NPC:
                    emit_casts(n + 1)
                # negated row-max of both halves in one DVE pass
                nmx = stat.tile([B, 2, 1], f32, tag="nmx")
                nc.vector.reduce_max(
                    out=nmx[:], in_=pab[:],
                    axis=mybir.AxisListType.X, negate=True)
                for t in range(2):
                    nc.scalar.activation(
                        att_t[:, (2 * sub + t) * B:(2 * sub + t + 1) * B],
                        pab[:, 1 - t, :],
                        mybir.ActivationFunctionType.Exp,
                        bias=nmx[:, 1 - t, 0:1])
                if sub == 1:
                    # output on the idle GPSIMD SWDGE ring so it never
                    # queues behind the input streams
                    nc.gpsimd.dma_start(
                        out=att[:, s * 512:(s + 1) * 512], in_=att_t[:])

        for f in reversed(frees):
            f()

    nc.compile()
    return nc


def _shard_inputs(left, right, mid):
    """Per-core [c, l, b]-contiguous fp16 hi + e5m2 lo shards; folds the
    softmax scale into mid."""
    in_maps = []
    for k in range(N_CORES):
        lo = 32 * k
        if lo < left.shape[2]:
            lsl = left[:, :, lo:lo + LW]
        else:
            lsl = right[:, :, lo - left.shape[2]:lo - left.shape[2] + LW]
        msl = mid[:, :, lo:lo + LW] * np.float32(SCALE)
        msl = np.ascontiguousarray(msl.transpose(1, 2, 0))
        lsl = np.ascontiguousarray(lsl.transpose(1, 2, 0))
        mh = msl.astype(np.float16)
        ml = (msl - mh.astype(np.float32)).astype(F8)
        lh = lsl.astype(np.float16)
        ll = (lsl - lh.astype(np.float32)).astype(F8)
        in_maps.append({"mh": mh, "ml": ml, "lh": lh, "ll": ll})
    return in_maps


def kernel(left, right, mid, sc00, sc01, sc10, sc11):
    global last_results
    left = np.asarray(left, dtype=np.float32)
    right = np.asarray(right, dtype=np.float32)
    mid = np.asarray(mid, dtype=np.float32)
    sc00 = np.asarray(sc00, dtype=np.float32)
    sc10 = np.asarray(sc10, dtype=np.float32)

    nc = build_program()
    in_maps = _shard_inputs(left, right, mid)
    trace = bool(int(os.environ.get("BASS_KERNEL_TRACE", "0")))
    last_results = run_bass_kernel_spmd(
        nc, in_maps, core_ids=list(range(N_CORES)), trace=trace,
    )

    # [k, b, n', t, o]
    att = np.stack([np.asarray(r["att"], dtype=np.float32)
                    for r in last_results.results])
    att = att.reshape(N_CORES, B, NPC, 2, B)
    att = att / att.sum(axis=4, keepdims=True)
    # -> [b, o(=c<128), n = k*NPC + n', t]
    attn = att.transpose(1, 4, 0, 2, 3).reshape(B, B, N_CORES * NPC, 2)

    Ls = sc00.shape[2]
    outs = []
    for sc in (sc00, sc10):
        out = np.zeros((B, C, Ls), np.float32)
        v = out.reshape(B, C, N_CORES * NPC, 3)
        v[:, :B, :, 0:2] = attn
        v[:, :, :, 2] = sc[:, :, :N_CORES * NPC]
        outs.append(out)
    return tuple(outs)


# revision 19
# speedup vs baseline: 1.0453x; 1.0181x over previous
"""Trainium2 Bass kernel for ContextualAttention (two_input=False path).

Math (B=128, C=512, n_iter=128, per iteration n):
    scores[n,b,o,0] = 10 * sum_c mid[b,c,2n]   * left_cat[o,c,2n+1]
    scores[n,b,o,1] = 10 * sum_c (mid[b,c,2n]*left_cat[o,c,2n]
                                  + mid[b,c,2n+1]*left_cat[o,c,2n+1])
    att = softmax(scores, axis=o)                                # [n,B,128,2]
    out0[b,c,3n+t] = att[n,b,c,t] (c<128, else 0); out0[b,c,3n+2] = sc00[b,c,n]
    out1 same with sc10. sc01/sc11 unused.

Sharding: data-parallel over n, 16 iterations per core (core k owns the
l-window [32k, 32k+32) of mid/left_cat).

Bottleneck model (from NTFF traces): the 16 per-core DMA engines cap at
~25 GB/s each (~400 GB/s aggregate), and the PE is moving-operand
fetch-bound at ~2 B/cycle, so both wire bytes and fetched bytes matter.

  - Wire format is 3 B/elem: fp16 hi plane + float8-e5m2 lo plane
    (lo = x - fp16(x), representable unscaled thanks to e5m2's 2^-16
    subnormals).  12.6 MB/core streams in ~33 us vs ~43 us for the fp32
    equivalent.
  - Scores: mh*lh (fp16 matmul) + [ml*lh8 + mh8*ll] (one fp8 DoubleRow
    matmul, two k-tiles per instruction).  mh8/lh8 are e5m2 casts of the
    hi planes done on-device on DVE (its 2x mode runs ~0.7us per
    iteration-slice).  Corrections are ~2^-11 relative so e5m2's coarse
    mantissa still leaves score error ~2^-14; measured end-to-end rel-err
    vs the fp64 reference is ~1.2e-3.
  - The correction accumulates directly into the main PSUM group (no
    scale-back pass, since the lo planes are unscaled).
  - DMA groups are ragged (8,8,8,4,2,2 l's): big chunks amortize
    descriptors early while the last chunks are small, so the final
    DMA -> cast -> matmul -> rowmax -> exp -> output chain after the
    last input byte is short.
  - Casts are emitted one iteration ahead of the row-max on the in-order
    DVE queue, and the PSUM pool is deep (12 bufs), so the PE is gated
    only by input arrival, never by the softmax drain.

Softmax: row-max (negated) via DVE feeds the exp activation bias on
ScalarE; exp writes fp16, and the host divides by the per-row sum (the max
shift cancels) and assembles the full outputs.
"""

import os
from functools import lru_cache

import ml_dtypes
import numpy as np

import concourse.bacc as bacc
import concourse.mybir as mybir
import concourse.tile as tile
from concourse.bass_utils import run_bass_kernel_spmd

N_CORES = 8
B = 128          # batch rows (= out partition) and also conv out channels o
C = 512          # contraction dim
NPC = 16         # iterations n per core
LW = 2 * NPC     # l-window per core (32)
GROUPS = (4, 8, 8, 8, 4)   # ragged l-group sizes (DMA/cast granularity)
NG = len(GROUPS)
GOFF = [sum(GROUPS[:i]) for i in range(NG + 1)]
SCALE = 10.0     # softmax scale, folded into mid on the host

F8 = ml_dtypes.float8_e5m2
F8_DT = mybir.dt.float8e5

# Results of the last run (exec_time_ns etc.), for the local test harness.
last_results = None


def _iter_group(n):
    """l-group index and local l-offset for iteration n (l = 2n)."""
    l = 2 * n
    for g in range(NG):
        if l < GOFF[g + 1]:
            return g, l - GOFF[g]
    raise ValueError(n)


@lru_cache(maxsize=1)
def build_program():
    """One SPMD program; all 8 cores run it on their own shard."""
    nc = bacc.Bacc(None, target_bir_lowering=False, debug=False)
    f32 = mybir.dt.float32
    f16 = mybir.dt.float16

    # Host-prepped layouts, per core:
    #   mh[c, l, b] = fp16(10 * mid[b, c, 32k + l])          [512, 32, 128]
    #   ml[c, l, b] = e5m2(10*mid - mh)                      [512, 32, 128]
    #   lh/ll same for left_cat[o, c, 32k + l]
    mh = nc.dram_tensor("mh", [C, LW, B], f16, kind="ExternalInput")
    ml = nc.dram_tensor("ml", [C, LW, B], F8_DT, kind="ExternalInput")
    lh = nc.dram_tensor("lh", [C, LW, B], f16, kind="ExternalInput")
    ll = nc.dram_tensor("ll", [C, LW, B], F8_DT, kind="ExternalInput")
    # att[b, n'*256 + t*128 + o] = exp(scores - rowmax)   (unnormalized)
    att = nc.dram_tensor("att", [B, NPC * 2 * B], f16, kind="ExternalOutput")

    # [c, cc, l, b] views: partition dim = c within a 128-chunk.
    mh_r = mh[:].rearrange("(cc c) l b -> c cc l b", cc=4)
    ml_r = ml[:].rearrange("(cc c) l b -> c cc l b", cc=4)
    lh_r = lh[:].rearrange("(cc c) l b -> c cc l b", cc=4)
    ll_r = ll[:].rearrange("(cc c) l b -> c cc l b", cc=4)

    DR = mybir.MatmulPerfMode.DoubleRow

    with tile.TileContext(nc) as tc:
        # All input tiles stay resident (96 KiB/partition total),
        # allocated individually (exact ragged sizes); freed LIFO below.
        mh_t, lh_t, m8_t, l8_t, frees = [], [], [], [], []
        for g in range(NG):
            gl = GROUPS[g]
            mhg, f0 = tc.tile([128, 4, gl, B], f16, name=f"mh{g}")
            lhg, f1 = tc.tile([128, 4, gl, B], f16, name=f"lh{g}")
            # fp8 pair tiles for DoubleRow: dim1 packs the two
            # stationary/moving planes contracted by one instruction.
            # m8: [0]=ml (lo, DMA), [1]=mh8 (hi cast)
            # l8: [0]=lh8 (hi cast), [1]=ll (lo, DMA)
            m8g, f2 = tc.tile([128, 2, 4, gl, B], F8_DT, name=f"m8{g}")
            l8g, f3 = tc.tile([128, 2, 4, gl, B], F8_DT, name=f"l8{g}")
            mh_t.append(mhg)
            lh_t.append(lhg)
            m8_t.append(m8g)
            l8_t.append(l8g)
            frees += [f0, f1, f2, f3]
        with (
            tc.tile_pool(name="stat", bufs=8) as stat,
            tc.tile_pool(name="attb", bufs=8) as attb,
            tc.tile_pool(name="ps", bufs=8, space="PSUM") as ps,
        ):
            # Input DMAs in l-group order, on two HWDGE rings: m tensors
            # on SP (idle engine — all issued up front; ring-full blocking
            # is harmless there), l tensors on ACT.  Critically, no up-front input
            # DMA beyond the first two groups is issued from the ACT
            # engine: dma_start blocks when the descriptor ring fills, and
            # anything queued behind it on that engine (the exps) would
            # stall until most of the stream has drained.  Later groups go
            # just-in-time (group g+2 at the start of group g's compute,
            # when g's descriptors have already drained from the ring), so
            # the issue never blocks and the exps behind it never wait.
            def emit_l_dma(g):
                sl = slice(GOFF[g], GOFF[g + 1])
                nc.scalar.dma_start(out=lh_t[g][:], in_=lh_r[:, :, sl, :])
                nc.scalar.dma_start(out=l8_t[g][:, 1], in_=ll_r[:, :, sl, :])

            # group 0's l tensors ride the SP ring: SP starts issuing
            # ~1.5us before ACT (which first loads its activation table),
            # and the first compute is gated on exactly this data.
            sl0 = slice(GOFF[0], GOFF[1])
            nc.sync.dma_start(out=mh_t[0][:], in_=mh_r[:, :, sl0, :])
            nc.sync.dma_start(out=m8_t[0][:, 0], in_=ml_r[:, :, sl0, :])
            nc.sync.dma_start(out=lh_t[0][:], in_=lh_r[:, :, sl0, :])
            nc.sync.dma_start(out=l8_t[0][:, 1], in_=ll_r[:, :, sl0, :])
            for g in range(1, NG):
                sl = slice(GOFF[g], GOFF[g + 1])
                nc.sync.dma_start(out=mh_t[g][:], in_=mh_r[:, :, sl, :])
                nc.sync.dma_start(out=m8_t[g][:, 0], in_=ml_r[:, :, sl, :])
            emit_l_dma(1)
            emit_l_dma(2)
            emit_l_dma(3)

            def emit_casts(n):
                # On-device e5m2 casts of the hi planes for iteration n
                # (DoubleRow needs both operands fp8).  Both on DVE — its
                # 2x mode runs them ~3x faster than ACT Copy.
                g, lo = _iter_group(n)
                lp = slice(lo, lo + 2)
                nc.vector.tensor_copy(
                    out=m8_t[g][:, 1, :, lp, :], in_=mh_t[g][:, :, lp, :])
                nc.vector.tensor_copy(
                    out=l8_t[g][:, 0, :, lp, :], in_=lh_t[g][:, :, lp, :])

            # The DVE program order is c0, c1, r0, c2, r1, ...: iteration
            # n's casts are emitted before iteration n-1's row-max, so the
            # PE (which only waits on casts) stays one iteration ahead of
            # the PE->reduce->cast dependency chain and never drains.
            emit_casts(0)
            group_started = set()
            for n in range(NPC):
                g, lo = _iter_group(n)
                if g not in group_started:
                    group_started.add(g)
                    # groups 1..3 are issued up front; the rest go
                    # just-in-time, one group per group-start from g=1 on
                    if g >= 1 and g + 3 < NG:
                        emit_l_dma(g + 3)
                mhg, lhg, m8g, l8g = mh_t[g], lh_t[g], m8_t[g], l8_t[g]
                l0, l1 = lo, lo + 1
                s, sub = divmod(n, 2)
                if sub == 0:
                    att_t = attb.tile([B, 4 * B], f16, tag="att")
                # psum [128, 2, 128]: [:,0]=t1 scores, [:,1]=t0 scores
                pab = ps.tile([B, 2, B], f32, tag="ps", name=f"pab{n}")
                for cc in range(4):
                    # fp16 main: stationary M(l0) x moving [L(l0)|L(l1)]
                    # writes [t1 | t0] at once; order mm,DR,DR,mm keeps
                    # ldweights hidden under the moving passes.
                    nc.tensor.matmul(
                        pab[:, :, :], mhg[:, cc, l0, :],
                        lhg[:, cc, l0:l0 + 2, :],
                        start=(cc == 0), stop=False)
                    # fp8 corrections: ml*lh8 + mh8*ll in one DoubleRow
                    nc.tensor.matmul(
                        pab[:, :, :], m8g[:, :, cc, l0, :],
                        l8g[:, :, cc, l0:l0 + 2, :],
                        start=False, stop=False, perf_mode=DR)
                    nc.tensor.matmul(
                        pab[:, 0, :], m8g[:, :, cc, l1, :],
                        l8g[:, :, cc, l1, :],
                        start=False, stop=False, perf_mode=DR)
                    # t1 second term: M(l1) x L(l1)
                    nc.tensor.matmul(
                        pab[:, 0, :], mhg[:, cc, l1, :],
                        lhg[:, cc, l1, :],
                        start=False, stop=(cc == 3))
                if n + 1 < NPC:
                    emit_casts(n + 1)
                # negated row-max of both halves in one DVE pass
                nmx = stat.tile([B, 2, 1], f32, tag="nmx")
                nc.vector.reduce_max(
                    out=nmx[:], in_=pab[:],
                    axis=mybir.AxisListType.X, negate=True)
                for t in range(2):
                    nc.scalar.activation(
                        att_t[:, (2 * sub + t) * B:(2 * sub + t + 1) * B],
                        pab[:, 1 - t, :],
                        mybir.ActivationFunctionType.Exp,
                        bias=nmx[:, 1 - t, 0:1])
                if sub == 1:
                    # output on the idle GPSIMD SWDGE ring so it never
                    # queues behind the input streams
                    nc.gpsimd.dma_start(
                        out=att[:, s * 512:(s + 1) * 512], in_=att_t[:])

        for f in reversed(frees):
            f()

    nc.compile()
    return nc


def _shard_inputs(left, right, mid):
    """Per-core [c, l, b]-contiguous fp16 hi + e5m2 lo shards; folds the
    softmax scale into mid."""
    in_maps = []
    for k in range(N_CORES):
        lo = 32 * k
        if lo < left.shape[2]:
            lsl = left[:, :, lo:lo + LW]
        else:
            lsl = right[:, :, lo - left.shape[2]:lo - left.shape[2] + LW]
        msl = mid[:, :, lo:lo + LW] * np.float32(SCALE)
        msl = np.ascontiguousarray(msl.transpose(1, 2, 0))
        lsl = np.ascontiguousarray(lsl.transpose(1, 2, 0))
        mh = msl.astype(np.float16)
        ml = (msl - mh.astype(np.float32)).astype(F8)
        lh = lsl.astype(np.float16)
        ll = (lsl - lh.astype(np.float32)).astype(F8)
        in_maps.append({"mh": mh, "ml": ml, "lh": lh, "ll": ll})
    return in_maps


def kernel(left, right, mid, sc00, sc01, sc10, sc11):
    global last_results
    left = np.asarray(left, dtype=np.float32)
    right = np.asarray(right, dtype=np.float32)
    mid = np.asarray(mid, dtype=np.float32)
    sc00 = np.asarray(sc00, dtype=np.float32)
    sc10 = np.asarray(sc10, dtype=np.float32)

    nc = build_program()
    in_maps = _shard_inputs(left, right, mid)
    trace = bool(int(os.environ.get("BASS_KERNEL_TRACE", "0")))
    last_results = run_bass_kernel_spmd(
        nc, in_maps, core_ids=list(range(N_CORES)), trace=trace,
    )

    # [k, b, n', t, o]
    att = np.stack([np.asarray(r["att"], dtype=np.float32)
                    for r in last_results.results])
    att = att.reshape(N_CORES, B, NPC, 2, B)
    att = att / att.sum(axis=4, keepdims=True)
    # -> [b, o(=c<128), n = k*NPC + n', t]
    attn = att.transpose(1, 4, 0, 2, 3).reshape(B, B, N_CORES * NPC, 2)

    Ls = sc00.shape[2]
    outs = []
    for sc in (sc00, sc10):
        out = np.zeros((B, C, Ls), np.float32)
        v = out.reshape(B, C, N_CORES * NPC, 3)
        v[:, :B, :, 0:2] = attn
        v[:, :, :, 2] = sc[:, :, :N_CORES * NPC]
        outs.append(out)
    return tuple(outs)
